# revision 1
# baseline (speedup 1.0000x reference)
"""Trainium2 Bass kernel for nn_DecoderBlock (upsample+merge+LN+2x Mamba).

Self-contained: builds/compiles the Bass program on first call (cached),
shards batch B=8 across 8 NeuronCores (data-parallel, no collectives),
runs via run_bass_kernel_spmd, reassembles the full (8,1024,512) output.
"""
import numpy as np

from contextlib import ExitStack

import concourse.bass as bass
import concourse.mybir as mybir
import concourse.tile as tile
from concourse.masks import make_identity

F32 = mybir.dt.float32
F32R = mybir.dt.float32r
AF = mybir.ActivationFunctionType
OP = mybir.AluOpType
AX = mybir.AxisListType

D, T, TS = 512, 512, 1024
DI, DS, DTR, K, NL = 1024, 16, 32, 4, 2
L = TS
P = 128
NG = DI // P             # 8 d-groups
NGH = NG // 2            # 4 d-groups per half
EPS = 1e-5
SILU_VIA_SIGMOID = False
MM_DT = F32  # set to bf16 for fast matmuls
HT_BUFS = 2
BUILD_PARTS = 'full'  # 'stageA' | 'p1' | 'full'
SCN_BUFS = 2
BCP_BUFS = 3
ENG_MODE = 2  # 0: alternate all; 1: b->Pool, hc->Pool, yadd->DVE; 2: b->Pool only   # set True for CoreSim runs (no Silu in interp)


def r32(ap):
    return ap  # plain fp32 matmuls (fp32r needs rounded producers)


def build(nc):
    def din(name, shape):
        return nc.dram_tensor(name, shape, F32, kind="ExternalInput").ap()

    x_d = din("x", [T, D])
    skip_d = din("skip", [TS, D])
    upw_d = din("up_w", [D, D * K])          # (d, (o,k)) flattened
    upb_d = din("up_b", [D, 1])
    mw_d = din("merge_w", [D, 2 * D])
    mb_d = din("merge_b", [D, 1])
    lnw_d = din("ln_w", [D, 1])
    lnb_d = din("ln_b", [D, 1])
    iw_d = din("in_proj_w", [NL, 2 * DI, D])
    cw_d = din("conv_w", [NL, DI, K])
    cb_d = din("conv_b", [NL, DI, 1])
    xw_d = din("x_proj_w", [NL, DTR + 2 * DS, DI])
    dw_d = din("dt_proj_w", [NL, DI, DTR])
    db_d = din("dt_proj_b", [NL, DI, 1])
    alog_d = din("A_log", [NL, DI, DS])
    dpar_d = din("D_param", [NL, DI, 1])
    ow_d = din("out_proj_w", [NL, D, DI])
    out_d = nc.dram_tensor("out", [L, D], F32, kind="ExternalOutput").ap()

    # DRAM scratch for per-layer spills
    dt_s = nc.dram_tensor("dt_scratch", [NG, P, L], F32).ap()
    du_s = nc.dram_tensor("du_scratch", [NG, P, L], F32).ap()
    zs_s = nc.dram_tensor("zs_scratch", [NG, P, L], F32).ap()
    xc_s = nc.dram_tensor("xc_scratch", [NG, P, L], F32).ap()
    bc_s = nc.dram_tensor("bc_scratch", [2, DS, L], F32).ap()
    st_s = nc.dram_tensor("st_scratch", [2, L], F32).ap()
    u_s = nc.dram_tensor("u_scratch", [NL, 4, P, L], MM_DT).ap()
    ow_s = nc.dram_tensor("ow_scratch", [NL, NG, P, D], MM_DT).ap()

    evict_rr = [0]

    def evict(dst, src, engine=None):
        if engine is None:
            engine = ("scalar", "vector")[evict_rr[0] % 2]
            evict_rr[0] += 1
        if engine == "scalar":
            nc.scalar.copy(dst, src)
        elif engine == "vector":
            nc.vector.tensor_copy(dst, src)
        else:
            nc.gpsimd.tensor_copy(dst, src)

    with tile.TileContext(nc) as tc, ExitStack() as ctx:
        const = ctx.enter_context(tc.tile_pool(name="const", bufs=1))
        ident = const.tile([P, P], F32, tag="ident", name="ident")
        make_identity(nc, ident)
        ident16 = const.tile([P, P], mybir.dt.bfloat16, tag="id16",
                             name="id16")
        nc.vector.tensor_copy(ident16[:], ident[:])

        # ================= stage A: upsample + merge + LN =================
        with ExitStack() as sctx:
            ldp = sctx.enter_context(tc.tile_pool(name="Aload", bufs=3))
            ptr = sctx.enter_context(tc.tile_pool(name="Aptr", bufs=2,
                                                  space="PSUM"))
            xTp = sctx.enter_context(tc.tile_pool(name="xTp", bufs=1))
            xT = [xTp.tile([P, T + 2], MM_DT, tag=f"xT{c}", name=f"xT{c}")
                  for c in range(4)]
            for c in range(4):
                nc.vector.memset(xT[c][:, 0:1], 0.0)
                nc.vector.memset(xT[c][:, T + 1:T + 2], 0.0)
            skT = [xTp.tile([P, TS], MM_DT, tag=f"skT{c}", name=f"skT{c}") for c in range(4)]

            def transpose_in(dst_tiles, src_dram, rows, dst_col0=0):  # noqa
                for rb in range(rows // P):
                    ld = ldp.tile([P, D], F32, tag="ld", name="ld")
                    nc.sync.dma_start(ld[:], src_dram[rb * P:(rb + 1) * P, :])
                    for cb_ in range(4):
                        ps = ptr.tile([P, P], F32, tag="ps", name="ps")
                        nc.tensor.transpose(
                            ps[:], ld[:, cb_ * P:(cb_ + 1) * P], ident[:])
                        evict(dst_tiles[cb_][:, dst_col0 + rb * P:
                                             dst_col0 + (rb + 1) * P], ps[:])

            transpose_in(xT, x_d, T, dst_col0=1)
            transpose_in(skT, skip_d, TS)

            # ---- upsample ----
            upwp = sctx.enter_context(tc.tile_pool(name="upwp", bufs=1))
            upw_sb = upwp.tile([P, 4 * D * K], MM_DT, tag="upw", name="upw")
            for c in range(4):
                if MM_DT is F32:
                    nc.sync.dma_start(upw_sb[:, c * D * K:(c + 1) * D * K],
                                      upw_d[c * P:(c + 1) * P, :])
                else:
                    uw = ldp.tile([P, D * K], F32, tag="uwst", name="uwst")
                    nc.sync.dma_start(uw[:], upw_d[c * P:(c + 1) * P, :])
                    nc.scalar.copy(upw_sb[:, c * D * K:(c + 1) * D * K],
                                   uw[:])
            upb_c = const.tile([P, 4], F32, tag="upb", name="upb")
            nc.sync.dma_start(upb_c[:].rearrange("p (a o) -> p a o", o=1),
                              upb_d[:].rearrange("(a p) o -> p a o", p=P))
            xuTp = sctx.enter_context(tc.tile_pool(name="xuTp", bufs=1))
            xuT = [xuTp.tile([P, TS], MM_DT, tag=f"xuT{c}", name=f"xuT{c}") for c in range(4)]
            pup = sctx.enter_context(tc.tile_pool(name="pup", bufs=4,
                                                  space="PSUM"))
            wv = upw_sb[:].rearrange("d (c o k) -> d c o k", c=4, k=K)
            for m in range(4):
                pe_ = pup.tile([P, T], F32, tag="pup", name="pup")
                po_ = pup.tile([P, T], F32, tag="pup", name="pup")
                for kc in range(4):
                    lhs_e = wv[:, kc, m * P:(m + 1) * P, 1]
                    lhs_o = wv[:, kc, m * P:(m + 1) * P, 2]
                    nc.tensor.matmul(pe_[:], r32(lhs_e),
                                     r32(xT[kc][:, 1:T + 1]),
                                     start=(kc == 0), stop=False)
                    nc.tensor.matmul(po_[:], r32(lhs_o),
                                     r32(xT[kc][:, 1:T + 1]),
                                     start=(kc == 0), stop=False)
                for kc in range(4):
                    lhs_e = wv[:, kc, m * P:(m + 1) * P, 3]
                    lhs_o = wv[:, kc, m * P:(m + 1) * P, 0]
                    nc.tensor.matmul(pe_[:], r32(lhs_e),
                                     r32(xT[kc][:, 0:T]), start=False,
                                     stop=(kc == 3))
                    nc.tensor.matmul(po_[:], r32(lhs_o),
                                     r32(xT[kc][:, 2:T + 2]), start=False,
                                     stop=(kc == 3))
                ev = xuT[m][:].rearrange("p (t two) -> p t two", two=2)
                nc.scalar.activation(ev[:, :, 0], pe_[:], AF.Identity,
                                     bias=upb_c[:, m:m + 1])
                nc.scalar.activation(ev[:, :, 1], po_[:], AF.Identity,
                                     bias=upb_c[:, m:m + 1])

            # ---- merge ----
            mwTp = sctx.enter_context(tc.tile_pool(name="mwTp", bufs=1))
            mwT = [mwTp.tile([P, D], MM_DT, tag=f"mwT{c}", name=f"mwT{c}") for c in range(8)]
            for rb in range(4):
                ld = ldp.tile([P, 2 * D], F32, tag="mwld", name="mwld")
                nc.sync.dma_start(ld[:], mw_d[rb * P:(rb + 1) * P, :])
                for cb_ in range(8):
                    ps = ptr.tile([P, P], F32, tag="ps", name="ps")
                    nc.tensor.transpose(ps[:], ld[:, cb_ * P:(cb_ + 1) * P],
                                        ident[:])
                    evict(mwT[cb_][:, rb * P:(rb + 1) * P], ps[:])
            mb_c = const.tile([P, 4], F32, tag="mbc", name="mbc")
            nc.sync.dma_start(mb_c[:].rearrange("p (a o) -> p a o", o=1),
                              mb_d[:].rearrange("(a p) o -> p a o", p=P))
            cat = xuT + skT
            mTp = sctx.enter_context(tc.tile_pool(name="mTp", bufs=1))
            mT = [mTp.tile([P, L], F32, tag=f"mT{c}", name=f"mT{c}") for c in range(4)]
            for m in range(4):
                for n in range(2):
                    ps = pup.tile([P, T], F32, tag="pup", name="pup")
                    for kc in range(8):
                        nc.tensor.matmul(
                            ps[:], r32(mwT[kc][:, m * P:(m + 1) * P]),
                            r32(cat[kc][:, n * T:(n + 1) * T]),
                            start=(kc == 0), stop=(kc == 7))
                    nc.scalar.activation(mT[m][:, n * T:(n + 1) * T], ps[:],
                                         AF.Identity, bias=mb_c[:, m:m + 1])

            # ---- LayerNorm over channels ----
            ones = const.tile([P, 1], F32, tag="ones", name="ones")
            nc.vector.memset(ones[:], 1.0)
            lnw_c = const.tile([P, 4], F32, tag="lnw", name="lnw")
            nc.sync.dma_start(lnw_c[:].rearrange("p (a o) -> p a o", o=1),
                              lnw_d[:].rearrange("(a p) o -> p a o", p=P))
            lnb_c = const.tile([P, 4], F32, tag="lnb", name="lnb")
            nc.sync.dma_start(lnb_c[:].rearrange("p (a o) -> p a o", o=1),
                              lnb_d[:].rearrange("(a p) o -> p a o", p=P))
            statp = sctx.enter_context(tc.tile_pool(name="statp", bufs=1))
            mu_r = statp.tile([1, L], F32, tag="mu", name="mu")
            s2_r = statp.tile([1, L], F32, tag="s2", name="s2")
            mu2 = statp.tile([1, L], F32, tag="mu2", name="mu2")
            inv_r = statp.tile([1, L], F32, tag="inv", name="inv")
            for n in range(2):
                ps = pup.tile([1, T], F32, tag="pln", name="pln", bufs=2)
                ps2 = pup.tile([1, T], F32, tag="pln", name="pln", bufs=2)
                for m in range(4):
                    nc.tensor.matmul(ps[:], r32(ones[:]),
                                     r32(mT[m][:, n * T:(n + 1) * T]),
                                     start=(m == 0), stop=(m == 3))
                for m in range(4):
                    sq = ldp.tile([P, T], F32, tag="sq", name="sq")
                    nc.scalar.square(sq[:], mT[m][:, n * T:(n + 1) * T])
                    nc.tensor.matmul(ps2[:], r32(ones[:]), r32(sq[:]),
                                     start=(m == 0), stop=(m == 3))
                nc.scalar.mul(mu_r[:, n * T:(n + 1) * T], ps[:], 1.0 / D)
                nc.scalar.mul(s2_r[:, n * T:(n + 1) * T], ps2[:], 1.0 / D)
            nc.vector.tensor_tensor(mu2[:], mu_r[:], mu_r[:], OP.mult)
            nc.vector.tensor_tensor(s2_r[:], s2_r[:], mu2[:], OP.subtract)
            nc.vector.tensor_scalar_add(s2_r[:], s2_r[:], EPS)
            nc.vector.reciprocal(s2_r[:], s2_r[:])
            nc.scalar.sqrt(inv_r[:], s2_r[:])
            nc.sync.dma_start(st_s[0].unsqueeze(0), mu_r[:])
            nc.sync.dma_start(st_s[1].unsqueeze(0), inv_r[:])
            mu_b = ldp.tile([P, L], F32, tag="mub", name="mub")
            inv_b = ldp.tile([P, L], F32, tag="invb", name="invb")
            nc.sync.dma_start(mu_b[:], st_s[0].unsqueeze(0).broadcast_to([P, L]))
            nc.sync.dma_start(inv_b[:], st_s[1].unsqueeze(0).broadcast_to([P, L]))
            for m in range(4):
                nc.vector.tensor_tensor(mT[m][:], mT[m][:], mu_b[:],
                                        OP.subtract)
                nc.vector.tensor_tensor(mT[m][:], mT[m][:], inv_b[:], OP.mult)
                u0 = ldp.tile([P, L], MM_DT, tag="u0", name="u0")
                nc.scalar.activation(u0[:], mT[m][:], AF.Identity,
                                     scale=lnw_c[:, m:m + 1],
                                     bias=lnb_c[:, m:m + 1])
                nc.sync.dma_start(u_s[0, m], u0[:])

        # ================= Mamba layers =================
        if BUILD_PARTS == 'stageA':
            return
        for li in range(NL):
            last = li == NL - 1
            with ExitStack() as lctx:
                # ---- per-layer constants ----
                cst = lctx.enter_context(tc.tile_pool(name=f"cst{li}", bufs=1))
                cw_c = cst.tile([P, NG * K], F32, tag="cw", name="cw")
                nc.sync.dma_start(
                    cw_c[:].rearrange("p (g k) -> p g k", k=K),
                    cw_d[li].rearrange("(g p) k -> p g k", p=P))
                cb_c = cst.tile([P, NG], F32, tag="cb", name="cb")
                nc.sync.dma_start(
                    cb_c[:].rearrange("p (g o) -> p g o", o=1),
                    cb_d[li].rearrange("(g p) o -> p g o", p=P))
                db_c = cst.tile([P, NG], F32, tag="db", name="db")
                nc.sync.dma_start(
                    db_c[:].rearrange("p (g o) -> p g o", o=1),
                    db_d[li].rearrange("(g p) o -> p g o", p=P))
                dpar_c = cst.tile([P, NG], F32, tag="dpar", name="dpar")
                nc.sync.dma_start(
                    dpar_c[:].rearrange("p (g o) -> p g o", o=1),
                    dpar_d[li].rearrange("(g p) o -> p g o", p=P))
                alog_c = cst.tile([P, NG * DS], F32, tag="alog", name="alog")
                nc.sync.dma_start(
                    alog_c[:].rearrange("p (g s) -> p g s", s=DS),
                    alog_d[li].rearrange("(g p) s -> p g s", p=P))
                A_c = cst.tile([P, NG * DS], F32, tag="Ac", name="Ac")
                nc.scalar.activation(A_c[:], alog_c[:], AF.Exp)
                db_n = cst.tile([P, NG], F32, tag="dbn", name="dbn")
                nc.vector.tensor_scalar_mul(db_n[:], db_c[:], -1.0)


                xdp = lctx.enter_context(tc.tile_pool(name=f"xdp{li}",
                                                      bufs=1))
                xd_all = xdp.tile([DTR + 2 * DS, L], F32, tag="xda",
                                  name="xda")
                dtrT = xd_all[0:DTR, :]

                # ======== phase 1: projections ========
                with ExitStack() as p1:
                    wload = p1.enter_context(
                        tc.tile_pool(name=f"wld{li}", bufs=2))
                    utp = p1.enter_context(
                        tc.tile_pool(name=f"utp{li}", bufs=1))
                    uT = [utp.tile([P, L], MM_DT, tag=f"ut{c}", name=f"ut{c}")
                          for c in range(4)]
                    for c in range(4):
                        nc.sync.dma_start(uT[c][:], u_s[li, c])
                    pw = p1.enter_context(
                        tc.tile_pool(name=f"pw{li}", bufs=2, space="PSUM"))
                    pmm = p1.enter_context(
                        tc.tile_pool(name=f"pmm{li}", bufs=3, space="PSUM"))
                    iwTp = p1.enter_context(
                        tc.tile_pool(name=f"iwT{li}", bufs=1))
                    iwT = [iwTp.tile([P, 2 * DI], MM_DT, tag=f"iwT{c}", name=f"iwT{c}")
                           for c in range(4)]
                    for rb in range(2 * DI // P):
                        ld = wload.tile([P, D], F32, tag="iwld", name="iwld")
                        nc.sync.dma_start(ld[:],
                                          iw_d[li, rb * P:(rb + 1) * P, :])
                        for cb_ in range(4):
                            ps = pw.tile([P, P], F32, tag="psw", name="psw")
                            nc.tensor.transpose(
                                ps[:], ld[:, cb_ * P:(cb_ + 1) * P], ident[:])
                            evict(iwT[cb_][:, rb * P:(rb + 1) * P], ps[:])
                    xwTp = p1.enter_context(
                        tc.tile_pool(name=f"xwT{li}", bufs=8))
                    NX = DTR + 2 * DS
                    xwT = [xwTp.tile([P, NX], MM_DT, tag="xwT", name="xwT")
                           for _ in range(8)]
                    ldx = wload.tile([NX, DI], F32, tag="xwld", name="xwld")
                    nc.sync.dma_start(ldx[:], xw_d[li])
                    for cb_ in range(8):
                        ps = pw.tile([P, P], F32, tag="psw", name="psw")
                        nc.tensor.transpose(
                            ps[:, 0:NX], ldx[:, cb_ * P:(cb_ + 1) * P],
                            ident[0:NX, 0:NX])
                        evict(xwT[cb_][:], ps[:, 0:NX])
                    dwTp = p1.enter_context(
                        tc.tile_pool(name=f"dwT{li}", bufs=8))
                    dwT = [dwTp.tile([DTR, P], MM_DT, tag="dwT", name="dwT")
                           for _ in range(8)]
                    for g in range(NG):
                        ld = wload.tile([P, DTR], F32, tag="dwld", name="dwld")
                        nc.sync.dma_start(ld[:],
                                          dw_d[li, g * P:(g + 1) * P, :])
                        ps = pw.tile([P, P], F32, tag="psw", name="psw")
                        nc.tensor.transpose(ps[0:DTR, 0:P], ld[:], ident[:])
                        evict(dwT[g][:], ps[0:DTR, 0:P])
                    owTp = p1.enter_context(
                        tc.tile_pool(name=f"owT{li}", bufs=1))
                    owT = [owTp.tile([P, D], MM_DT, tag=f"owT{c}",
                                     name=f"owT{c}") for c in range(NG)]
                    for rb in range(4):
                        ld = wload.tile([P, DI], F32, tag="owld", name="owld")
                        nc.sync.dma_start(ld[:],
                                          ow_d[li, rb * P:(rb + 1) * P, :])
                        for cb_ in range(8):
                            ps = pw.tile([P, P], F32, tag="psw", name="psw")
                            nc.tensor.transpose(
                                ps[:], ld[:, cb_ * P:(cb_ + 1) * P], ident[:])
                            evict(owT[cb_][:, rb * P:(rb + 1) * P], ps[:])
                    for g in range(NG):
                        nc.sync.dma_start(ow_s[li, g], owT[g][:])

                    # ---- in_proj xi-half + conv + silu -> xc -> spill ----
                    xpadp = p1.enter_context(
                        tc.tile_pool(name=f"xpad{li}", bufs=2))
                    xcp = p1.enter_context(
                        tc.tile_pool(name=f"xcp{li}", bufs=3))
                    pxp = [pmm.tile([NX, T], F32, tag="pxp", name="pxp")
                           for _ in range(2)]
                    for g in range(NG):
                        xpad = xpadp.tile([P, K - 1 + L], F32, tag="xpad", name="xpad")
                        nc.vector.memset(xpad[:, 0:K - 1], 0.0)
                        for n in range(2):
                            ps = pmm.tile([P, T], F32, tag="pmm", name="pmm")
                            for kc in range(4):
                                nc.tensor.matmul(
                                    ps[:],
                                    r32(iwT[kc][:, g * P:(g + 1) * P]),
                                    r32(uT[kc][:, n * T:(n + 1) * T]),
                                    start=(kc == 0), stop=(kc == 3))
                            nc.vector.tensor_copy(
                                xpad[:, K - 1 + n * T:K - 1 + (n + 1) * T],
                                ps[:])
                        xcg = xcp.tile([P, L], F32, tag="xcg", name="xcg")
                        xcg16 = (xcg if MM_DT is F32 else
                                 xcp.tile([P, L], MM_DT, tag="xc16",
                                          name="xc16"))
                        nc.vector.tensor_scalar_mul(xcg[:], xpad[:, 0:L],
                                                    cw_c[:, g * K:g * K + 1])
                        for kk in range(1, K):
                            nc.vector.scalar_tensor_tensor(
                                xcg[:], xpad[:, kk:kk + L],
                                cw_c[:, g * K + kk:g * K + kk + 1],
                                xcg[:], op0=OP.mult, op1=OP.add)
                        if SILU_VIA_SIGMOID:
                            sgt = xcp.tile([P, L], F32, tag="sgt", name="sgt")
                            nc.scalar.activation(xcg[:], xcg[:], AF.Identity,
                                                 bias=cb_c[:, g:g + 1])
                            nc.scalar.activation(sgt[:], xcg[:], AF.Sigmoid)
                            nc.vector.tensor_tensor(xcg[:], xcg[:], sgt[:],
                                                    OP.mult)
                        else:
                            nc.scalar.activation(xcg[:], xcg[:], AF.Silu,
                                                 bias=cb_c[:, g:g + 1])
                        nc.sync.dma_start(xc_s[g], xcg[:])
                        if MM_DT is not F32:
                            nc.scalar.copy(xcg16[:], xcg[:])
                        for n in range(2):
                            nc.tensor.matmul(
                                pxp[n][:], r32(xwT[g][:]),
                                r32(xcg16[:, n * T:(n + 1) * T]),
                                start=(g == 0), stop=(g == NG - 1),
                                skip_group_check=True)
                    for n in range(2):
                        nc.vector.tensor_copy(xd_all[:, n * T:(n + 1) * T],
                                              pxp[n][0:NX, :])
                    nc.vector.tensor_scalar_mul(
                        xd_all[DTR:DTR + DS, :], xd_all[DTR:DTR + DS, :],
                        -1.0)
                    nc.sync.dma_start(bc_s[0], xd_all[DTR:DTR + DS, :])
                    nc.sync.dma_start(bc_s[1], xd_all[DTR + DS:NX, :])

                    # ---- z-half: silu -> spill ----
                    for g in range(NG):
                        zt = xcp.tile([P, L], F32, tag="zt", name="zt")
                        for n in range(2):
                            ps = pmm.tile([P, T], F32, tag="pmm", name="pmm")
                            for kc in range(4):
                                nc.tensor.matmul(
                                    ps[:],
                                    r32(iwT[kc][:, DI + g * P:
                                                DI + (g + 1) * P]),
                                    r32(uT[kc][:, n * T:(n + 1) * T]),
                                    start=(kc == 0), stop=(kc == 3))
                            if SILU_VIA_SIGMOID:
                                sg2 = xcp.tile([P, T], F32, tag="sg2",
                                               name="sg2")
                                nc.scalar.activation(sg2[:], ps[:],
                                                     AF.Sigmoid)
                                nc.vector.tensor_tensor(
                                    zt[:, n * T:(n + 1) * T],
                                    ps[:], sg2[:], OP.mult)
                            else:
                                nc.scalar.activation(
                                    zt[:, n * T:(n + 1) * T], ps[:], AF.Silu)
                        nc.sync.dma_start(zs_s[g], zt[:])

                    # ---- dt_proj -> softplus ; dtu ; spill ----
                    if MM_DT is not F32:
                        dtr16 = xdp.tile([DTR, L], MM_DT, tag="dtr16",
                                         name="dtr16")
                        nc.vector.tensor_copy(dtr16[:], dtrT[:])
                    else:
                        dtr16 = dtrT
                    for g in range(NG):
                        dtg = xcp.tile([P, L], F32, tag="dtg", name="dtg")
                        for n in range(2):
                            ps = pmm.tile([P, T], F32, tag="pmm", name="pmm")
                            nc.tensor.matmul(
                                ps[:], r32(dwT[g][:]),
                                r32(dtr16[:, n * T:(n + 1) * T]),
                                start=True, stop=True)
                            nc.scalar.activation(
                                dtg[:, n * T:(n + 1) * T], ps[:],
                                AF.Sigmoid, scale=-1.0,
                                bias=db_n[:, g:g + 1])
                        nc.scalar.activation(dtg[:], dtg[:], AF.Ln)
                        xcg = xcp.tile([P, L], F32, tag="xcr", name="xcr")
                        nc.sync.dma_start(xcg[:], xc_s[g])
                        dug = xcp.tile([P, L], F32, tag="dug", name="dug")
                        nc.vector.tensor_tensor(dug[:], dtg[:], xcg[:],
                                                OP.mult)
                        nc.sync.dma_start(dt_s[g], dtg[:])
                        nc.sync.dma_start(du_s[g], dug[:])

                # ======== phase 2: fused scan + PSUM y-accum + out_proj ====
                if BUILD_PARTS == 'p1':
                    continue
                with ExitStack() as p2:
                    FL = NGH * L
                    scn = p2.enter_context(
                        tc.tile_pool(name=f"scn{li}", bufs=SCN_BUFS))
                    htp = p2.enter_context(
                        tc.tile_pool(name=f"htp{li}", bufs=HT_BUFS))
                    strm = p2.enter_context(
                        tc.tile_pool(name=f"strm{li}", bufs=1))
                    bcp = p2.enter_context(
                        tc.tile_pool(name=f"bcp{li}", bufs=BCP_BUFS))
                    gyp = p2.enter_context(
                        tc.tile_pool(name=f"gyp{li}", bufs=2))
                    pp2 = p2.enter_context(
                        tc.tile_pool(name=f"pp2{li}", bufs=8, space="PSUM"))
                    otp = p2.enter_context(
                        tc.tile_pool(name=f"otp{li}", bufs=2))
                    uacc = [otp.tile([P, D if last else T], F32, tag="ua",
                                     name="ua", bufs=8) for _ in range(8)]

                    for half in range(2):
                        g0 = half * NGH
                        dth_f = strm.tile([P, FL], F32, tag="dth",
                                          name="dth")
                        duh_f = strm.tile([P, FL], F32, tag="duh",
                                          name="duh")
                        for j in range(NGH):
                            nc.sync.dma_start(dth_f[:, j * L:(j + 1) * L],
                                              dt_s[g0 + j])
                            nc.sync.dma_start(duh_f[:, j * L:(j + 1) * L],
                                              du_s[g0 + j])
                        psum_y = [pp2.tile([P, T], F32, tag="py", name="py")
                                  for _ in range(8)]
                        for s in range(DS):
                            Bb = bcp.tile([P, L], F32, tag="Bb", name="Bb")
                            Cb = bcp.tile([P, L], F32, tag="Cb", name="Cb")
                            nc.sync.dma_start(
                                Bb[:],
                                bc_s[0, s].unsqueeze(0).broadcast_to([P, L]))
                            nc.sync.dma_start(
                                Cb[:],
                                bc_s[1, s].unsqueeze(0).broadcast_to([P, L]))
                            at = scn.tile([P, FL], F32, tag="at", name="at")
                            for j in range(NGH):
                                g = g0 + j
                                nc.scalar.activation(
                                    at[:, j * L:(j + 1) * L],
                                    dth_f[:, j * L:(j + 1) * L], AF.Exp,
                                    scale=A_c[:, g * DS + s:g * DS + s + 1])
                            atv = at[:].rearrange("p (j t) -> p j t", j=NGH)
                            nc.vector.memset(atv[:, 1:NGH, 0], 0.0)
                            bt = scn.tile([P, FL], F32, tag="bt", name="bt")
                            Bbv = Bb[:].unsqueeze(1).broadcast_to([P, NGH, L])
                            Cbv = Cb[:].unsqueeze(1).broadcast_to([P, NGH, L])
                            duv = duh_f[:].rearrange("p (j t) -> p j t",
                                                     j=NGH)
                            btv = bt[:].rearrange("p (j t) -> p j t", j=NGH)
                            nc.gpsimd.tensor_tensor(btv, duv, Bbv, OP.mult)
                            ht = htp.tile([P, FL], mybir.dt.bfloat16, tag="ht", name="ht")
                            nc.vector.tensor_tensor_scan(
                                ht[:], at[:], bt[:], 0.0, OP.mult, OP.add)
                            htv = ht[:].rearrange("p (j t) -> p j t", j=NGH)
                            if ENG_MODE == 2:
                                e_hc = nc.vector
                            elif ENG_MODE == 4:
                                e_hc = (nc.gpsimd if s % 4 == 3
                                        else nc.vector)
                            elif ENG_MODE == 5:
                                e_hc = (nc.gpsimd if s % 8 == 7
                                        else nc.vector)
                            else:
                                e_hc = (nc.vector if s % 2 == 0
                                        else nc.gpsimd)
                            e_hc.tensor_tensor(htv, htv, Cbv, OP.mult)
                            for ch in range(8):
                                nc.tensor.matmul(
                                    psum_y[ch][:], ident16[:],
                                    ht[:, ch * T:(ch + 1) * T],
                                    start=(s == 0), stop=(s == DS - 1),
                                    skip_group_check=True)
                        # gating (reads y from PSUM) into SBUF gy tiles
                        gys = []
                        for j in range(NGH):
                            g = g0 + j
                            zt = gyp.tile([P, L], F32, tag="ztg", name="ztg", bufs=1)
                            nc.sync.dma_start(zt[:], zs_s[g])
                            xcg = gyp.tile([P, L], F32, tag="xcg2",
                                           name="xcg2", bufs=1)
                            nc.sync.dma_start(xcg[:], xc_s[g])
                            gy = gyp.tile([P, L], F32, tag="gy", name="gy",
                                          bufs=4)
                            for n in range(2):
                                nc.vector.scalar_tensor_tensor(
                                    gy[:, n * T:(n + 1) * T],
                                    xcg[:, n * T:(n + 1) * T],
                                    dpar_c[:, g:g + 1],
                                    psum_y[j * 2 + n][:],
                                    op0=OP.mult, op1=OP.add)
                            nc.vector.tensor_tensor(gy[:], gy[:], zt[:],
                                                    OP.mult)
                            gys.append(gy)
                        # out_proj accumulation over j
                        pso = [pp2.tile([P, D if last else T], F32,
                                        tag="py", name="py")
                               for _ in range(8)]
                        for j in range(NGH):
                            g = g0 + j
                            if MM_DT is F32:
                                gy = gys[j]
                            else:
                                gy = gyp.tile([P, L], MM_DT, tag="gyc",
                                              name="gyc", bufs=2)
                                nc.scalar.copy(gy[:], gys[j][:])
                            owg = gyp.tile([P, D], MM_DT, tag="owg",
                                           name="owg")
                            nc.sync.dma_start(owg[:], ow_s[li, g])
                            if last:
                                for mt in range(8):
                                    nc.tensor.matmul(
                                        pso[mt][:],
                                        r32(gy[:, mt * P:(mt + 1) * P]),
                                        r32(owg[:]), start=(j == 0),
                                        stop=(j == NGH - 1),
                                        skip_group_check=True)
                            else:
                                for m in range(4):
                                    for n in range(2):
                                        nc.tensor.matmul(
                                            pso[m * 2 + n][:],
                                            r32(owg[:,
                                                    m * P:(m + 1) * P]),
                                            r32(gy[:, n * T:(n + 1) * T]),
                                            start=(j == 0),
                                            stop=(j == NGH - 1),
                                            skip_group_check=True)
                        # merge half contribution into SBUF accumulators
                        for ch in range(8):
                            if half == 0:
                                evict(uacc[ch][:], pso[ch][:])
                            else:
                                nc.vector.tensor_tensor(
                                    uacc[ch][:], uacc[ch][:], pso[ch][:],
                                    OP.add)
                    if last:
                        for mt in range(8):
                            nc.sync.dma_start(out_d[mt * P:(mt + 1) * P, :],
                                              uacc[mt][:])
                    else:
                        for m in range(4):
                            ot = otp.tile([P, L], MM_DT, tag="otn", name="otn")
                            for n in range(2):
                                nc.vector.tensor_copy(
                                    ot[:, n * T:(n + 1) * T],
                                    uacc[m * 2 + n][:])
                            nc.sync.dma_start(u_s[li + 1, m], ot[:])


_CACHE = {}


def _get_nc():
    if "nc" not in _CACHE:
        from concourse import bacc
        nc = bacc.Bacc("TRN2", target_bir_lowering=False, debug=False,
                       num_devices=8)
        build(nc)
        nc.compile()
        _CACHE["nc"] = nc
    return _CACHE["nc"]


def _prep(inputs, b):
    f = lambda a: np.ascontiguousarray(np.asarray(a), dtype=np.float32)
    return {
        "x": f(inputs["x"][b]),
        "skip": f(inputs["skip"][b]),
        "up_w": f(inputs["up_w"]).reshape(512, 2048),
        "up_b": f(inputs["up_b"]).reshape(512, 1),
        "merge_w": f(inputs["merge_w"]),
        "merge_b": f(inputs["merge_b"]).reshape(512, 1),
        "ln_w": f(inputs["ln_w"]).reshape(512, 1),
        "ln_b": f(inputs["ln_b"]).reshape(512, 1),
        "in_proj_w": f(inputs["in_proj_w"]),
        "conv_w": f(inputs["conv_w"]),
        "conv_b": f(inputs["conv_b"]).reshape(2, 1024, 1),
        "x_proj_w": f(inputs["x_proj_w"]),
        "dt_proj_w": f(inputs["dt_proj_w"]),
        "dt_proj_b": f(inputs["dt_proj_b"]).reshape(2, 1024, 1),
        "A_log": f(inputs["A_log"]),
        "D_param": f(inputs["D_param"]).reshape(2, 1024, 1),
        "out_proj_w": f(inputs["out_proj_w"]),
    }


def kernel(**inputs):
    from concourse.bass_utils import run_bass_kernel_spmd
    nc = _get_nc()
    B = int(np.asarray(inputs["x"]).shape[0])
    assert B == 8, f"expected B=8, got {B}"
    in_maps = [_prep(inputs, b) for b in range(B)]
    res = run_bass_kernel_spmd(nc, in_maps, list(range(8)))
    out = np.stack([res.results[b]["out"] for b in range(B)])
    return out.astype(np.float32)

